# revision 3
# baseline (speedup 1.0000x reference)
"""Trainium2 Bass kernel for the DEER-MLP spiking network (v2).

Network: x(4,32,196,384) -> FC1(384->1536) -> BatchNorm -> LIF(T=4) ->
FC2(1536->384) -> BatchNorm -> LIF -> spikes(4,32,196,384).

Math notes:
 - The reference's 10 DEER Newton iterations over T=4 steps converge to the
   exact sequential LIF recurrence; we compute that directly.
 - The pre-BN biases b1/b2 cancel inside BatchNorm (additive per-channel
   constants are removed by the mean subtraction), so they are dropped.
 - Both matmuls run as multi-pass fp16 with hi/lo fp16 limbs (exact to
   ~2^-22): FC1 = x_hi@w_hi + x_lo@w_hi + x_hi@w_lo; FC2 spikes are exact
   in fp16 so two passes (w_hi + w_lo) suffice.

Distribution: data-parallel over B across 8 cores (784 lanes/core).
BatchNorm statistics are the only cross-core coupling: a warmup dummy
AllReduce, four staged BN1 stat AllReduces (pipelined under FC1), and one
BN2 AllReduce.

v2 schedule (vs the phase-separated v1):
 - FC1 is h-tile-outer (stationary weights swept over 7 uniform 448-col
   moving chunks) so each h-channel finishes early; y1 never leaves SBUF.
 - BN1 stats: per-chunk sumsq via a Square activation written back into
   the dead PSUM bank (accum_out), per-h-tile sum via one DVE reduce.
   Stats AllReduce in 4 groups of 3 h-tiles, overlapped with FC1.
 - BN1 affine + LIF1 run on DVE in the shadow of FC1, one stats-group
   behind; the last group is processed in two m-slices so FC2 can start
   right after the final stats AllReduce.
 - FC2 uses s1 tiles as the stationary operand, producing y2 ROW-major
   ([m-rows, c]); BN2 stats are accumulated with DVE adds + a ones-matmul
   partition reduce; coeffs are broadcast back to 128 partitions with a
   rank-1 matmul.  LIF2 emits spikes row-major and DMAs straight to the
   output - no transpose phase at all.
"""

import numpy as np

import concourse.bass as bass
import concourse.mybir as mybir
import concourse.tile as tile
from concourse import bacc
from concourse.bass_utils import run_bass_kernel_spmd

F32 = mybir.dt.float32
F16 = mybir.dt.float16
AF = mybir.ActivationFunctionType
OP = mybir.AluOpType
AX = mybir.AxisListType

T, B, NN, C, H = 4, 32, 196, 384, 1536
NCORES = 8
BLOC = B // NCORES            # 4 batches per core
MLOC = BLOC * NN              # 784 lanes per core
R = T * MLOC                  # 3136 flattened (t, m) rows per core
NTOT = T * B * NN             # 25088 batchnorm samples per channel
KC = C // 128                 # 3 c-tiles
KH = H // 128                 # 12 h-tiles
EPS = 1e-5
P = 128

CH = 448                      # FC1 moving-operand chunk (7 * 448 = 3136)
NCH = R // CH
MT = 128                      # FC2 m-tile (rows per output group)
NMT = MLOC // MT              # 6 full m-tiles per t step
MTAIL = MLOC - NMT * MT       # 16 tail rows per t step
HGRP = 3                      # h-tiles per BN1 stats allreduce group
NGRP = KH // HGRP             # 4 staged allreduces


def _lif1_slice(nc, pool, y1a, s1a, a, dsc1, dsh1, m0, mlen, tg):
    """BN1 affine + LIF over [P, T, m0:m0+mlen] of h-tile a (in place on
    y1a); spikes to s1a fp16.  tg selects the scratch-tile tag set."""
    yv = y1a[:].rearrange("p (t m) -> p t m", t=T)
    nc.vector.tensor_scalar(
        yv[:, :, m0 : m0 + mlen], yv[:, :, m0 : m0 + mlen],
        dsc1[:, a : a + 1], dsh1[:, a : a + 1], OP.mult, OP.add)
    v = None
    for t in range(T):
        d = yv[:, t, m0 : m0 + mlen]
        if v is None:
            h = d
        else:
            ht = pool.tile([P, mlen], F32, tag=f"{tg}_h", name=f"{tg}h{a}_{t}")
            nc.vector.scalar_tensor_tensor(ht[:], v, 0.5, d, OP.mult, OP.add)
            h = ht[:]
        nc.vector.tensor_scalar(s1a[:, t, m0 : m0 + mlen], h, 1.0, None,
                                OP.is_ge)
        if t < T - 1:
            vt = pool.tile([P, mlen], F32, tag=f"{tg}_v", name=f"{tg}v{a}_{t}")
            nc.vector.scalar_tensor_tensor(vt[:], h, 1.0, h, OP.is_lt,
                                           OP.mult)
            v = vt[:]


def _bn_coeffs(nc, pool, stg, gt, bet, k, pp, tag, dsc_out, dsh_out):
    """From allreduced [pp, 2k] (sum || sumsq) write the fused affine
    coeffs: drive = y*dsc + dsh == 0.5*((y - mean)*rsqrt(var+eps)*g + be)."""
    mean = pool.tile([pp, k], F32, tag=f"{tag}_mean", name=f"{tag}_mean")
    nc.vector.tensor_scalar(mean[:], stg[:, 0:k], 1.0 / NTOT, None, OP.mult)
    var = pool.tile([pp, k], F32, tag=f"{tag}_var", name=f"{tag}_var")
    nc.vector.tensor_scalar(var[:], stg[:, k : 2 * k], 1.0 / NTOT, None,
                            OP.mult)
    msq = pool.tile([pp, k], F32, tag=f"{tag}_msq", name=f"{tag}_msq")
    nc.vector.tensor_tensor(msq[:], mean[:], mean[:], OP.mult)
    nc.vector.tensor_tensor(var[:], var[:], msq[:], OP.subtract)
    nc.vector.tensor_scalar(var[:], var[:], EPS, None, OP.add)
    std = pool.tile([pp, k], F32, tag=f"{tag}_std", name=f"{tag}_std")
    nc.scalar.activation(std[:], var[:], AF.Sqrt, bias=0.0, scale=1.0)
    rstd = pool.tile([pp, k], F32, tag=f"{tag}_rstd", name=f"{tag}_rstd")
    nc.vector.reciprocal(rstd[:], std[:])
    nc.vector.tensor_tensor(dsc_out, rstd[:], gt, OP.mult)
    nc.vector.tensor_tensor(dsh_out, mean[:], dsc_out, OP.mult)
    nc.vector.tensor_tensor(dsh_out, bet, dsh_out, OP.subtract)
    nc.vector.tensor_scalar(dsc_out, dsc_out, 0.5, None, OP.mult)
    nc.vector.tensor_scalar(dsh_out, dsh_out, 0.5, None, OP.mult)


def _build():
    nc = bacc.Bacc("TRN2", target_bir_lowering=False, debug=False,
                   num_devices=NCORES)

    xh_d = nc.dram_tensor("xthi", [KC, P, R], F16, kind="ExternalInput")
    xl_d = nc.dram_tensor("xtlo", [KC, P, R], F16, kind="ExternalInput")
    w1h_d = nc.dram_tensor("w1thi", [KC, P, H], F16, kind="ExternalInput")
    w1l_d = nc.dram_tensor("w1tlo", [KC, P, H], F16, kind="ExternalInput")
    w2h_d = nc.dram_tensor("w2thi", [KH, P, C], F16, kind="ExternalInput")
    w2l_d = nc.dram_tensor("w2tlo", [KH, P, C], F16, kind="ExternalInput")
    g1_d = nc.dram_tensor("g1", [H], F32, kind="ExternalInput")
    be1_d = nc.dram_tensor("be1", [H], F32, kind="ExternalInput")
    g2_d = nc.dram_tensor("g2", [C], F32, kind="ExternalInput")
    be2_d = nc.dram_tensor("be2", [C], F32, kind="ExternalInput")
    out_d = nc.dram_tensor("out", [R, C], F32, kind="ExternalOutput")

    groups = [list(range(NCORES))]

    with tile.TileContext(nc) as tc:
        with (
            tc.tile_pool(name="const", bufs=1) as const,
            tc.tile_pool(name="dram", bufs=1, space="DRAM") as dram,
        ):
            def colvec(dst_k, src):
                t_ = const.tile([P, dst_k], F32, name=f"cv_{src.name}",
                                tag=f"cv_{src.name}")
                nc.sync.dma_start(
                    t_[:], src.ap().rearrange("(a p) -> p a", p=P))
                return t_

            def rowvec(src, n):
                t_ = const.tile([1, n], F32, name=f"rv_{src.name}",
                                tag=f"rv_{src.name}")
                nc.sync.dma_start(t_[:], src.ap().rearrange("(a c) -> a c",
                                                            a=1))
                return t_

            g1t, be1t = colvec(KH, g1_d), colvec(KH, be1_d)
            g2r, be2r = rowvec(g2_d, C), rowvec(be2_d, C)

            onesc = const.tile([P, 1], F32)
            nc.vector.memset(onesc[:], 1.0)
            ones1r = const.tile([1, P], F32)
            nc.vector.memset(ones1r[:], 1.0)

            # Warmup collective: absorbs CC-firmware cold start + launch
            # skew in the shadow of the input DMAs.  Its result is folded
            # (times zero) into ones1r so it cannot be dead-code-eliminated.
            bar_s = const.tile([1, 8], F32)
            nc.vector.memset(bar_s[:], 1.0)
            bar_in = dram.tile([1, 8], F32, tag="bar_in", name="bar_in")
            bar_out = dram.tile([1, 8], F32, tag="bar_out", name="bar_out")
            nc.gpsimd.dma_start(bar_in[:], bar_s[:])
            nc.gpsimd.collective_compute(
                "AllReduce", OP.add, replica_groups=groups,
                ins=[bar_in.opt()], outs=[bar_out.opt()])
            bar_r = const.tile([1, 8], F32)
            nc.gpsimd.dma_start(bar_r[:], bar_out[:])
            nc.vector.scalar_tensor_tensor(
                ones1r[0:1, 0:8], bar_r[:], 0.0, ones1r[0:1, 0:8],
                OP.mult, OP.add)

            # persistent across phases
            s1 = [const.tile([P, T, MLOC], F16, tag=f"s1_{k}",
                             name=f"s1_{k}") for k in range(KH)]
            asum1 = const.tile([P, KH], F32)
            asq1 = const.tile([P, KH, NCH], F32)
            dsc1 = const.tile([P, KH], F32)
            dsh1 = const.tile([P, KH], F32)

            st_in = [dram.tile([P, 2 * HGRP], F32, tag=f"sti{g}",
                               name=f"sti{g}") for g in range(NGRP)]
            st_out = [dram.tile([P, 2 * HGRP], F32, tag=f"sto{g}",
                                name=f"sto{g}") for g in range(NGRP)]
            st2_in = dram.tile([1, 2 * C], F32)
            st2_out = dram.tile([1, 2 * C], F32)

            # ---- phase A: FC1 (h-outer) + staged BN1 stats + LIF1 ------
            with (
                tc.tile_pool(name="pax", bufs=1) as pax,
                tc.tile_pool(name="py1", bufs=4) as py1,
                tc.tile_pool(name="plif", bufs=2) as plif,
                tc.tile_pool(name="ps_mm", bufs=1, space="PSUM") as ps_mm,
            ):
                w1h = pax.tile([P, KC, H], F16)
                nc.sync.dma_start(w1h[:],
                                  w1h_d.ap().rearrange("k p h -> p k h"))
                xh = pax.tile([P, KC, R], F16)
                nc.sync.dma_start(xh[:], xh_d.ap().rearrange("k p r -> p k r"))
                xl = pax.tile([P, KC, R], F16)
                nc.sync.dma_start(xl[:], xl_d.ap().rearrange("k p r -> p k r"))
                w1l = pax.tile([P, KC, H], F16)
                nc.sync.dma_start(w1l[:],
                                  w1l_d.ap().rearrange("k p h -> p k h"))

                y1 = [None] * KH
                lif_pending = []
                for a in range(KH):
                    pss = [ps_mm.tile([P, CH], F32, tag=f"mm{c}",
                                      name=f"ps{a}_{c}") for c in range(NCH)]
                    idx = 0
                    for wt, xt in ((w1h, xh), (w1l, xh), (w1h, xl)):
                        for k in range(KC):
                            for c in range(NCH):
                                nc.tensor.matmul(
                                    pss[c][:],
                                    wt[:, k, a * P : (a + 1) * P],
                                    xt[:, k, c * CH : (c + 1) * CH],
                                    start=(idx == 0),
                                    stop=(idx == 8),
                                )
                            idx += 1
                    y1a = py1.tile([P, R], F32, tag="y1", name=f"y1_{a}")
                    y1[a] = y1a
                    for c in range(NCH):
                        # evacuate; then sumsq stats via a Square written
                        # back into the (dead) psum bank with accum_out
                        nc.scalar.activation(
                            y1a[:, c * CH : (c + 1) * CH], pss[c][:],
                            AF.Identity, bias=0.0, scale=1.0)
                        nc.scalar.activation(
                            pss[c][:], pss[c][:], AF.Square,
                            bias=0.0, scale=1.0,
                            accum_out=asq1[:, a, c : c + 1])
                    nc.vector.tensor_reduce(
                        asum1[:, a : a + 1], y1a[:], AX.X, OP.add)

                    if a % HGRP == HGRP - 1:
                        g = a // HGRP
                        a0 = g * HGRP
                        stg = const.tile([P, 2 * HGRP], F32,
                                         tag=f"stg{g}", name=f"stg{g}")
                        nc.vector.tensor_copy(
                            stg[:, 0:HGRP], asum1[:, a0 : a0 + HGRP])
                        nc.vector.tensor_reduce(
                            stg[:, HGRP : 2 * HGRP],
                            asq1[:, a0 : a0 + HGRP, :], AX.X, OP.add)
                        nc.gpsimd.dma_start(st_in[g][:], stg[:])
                        nc.gpsimd.collective_compute(
                            "AllReduce", OP.add, replica_groups=groups,
                            ins=[st_in[g].opt()], outs=[st_out[g].opt()])
                        stga = const.tile([P, 2 * HGRP], F32,
                                          tag=f"stga{g}", name=f"stga{g}")
                        nc.gpsimd.dma_start(stga[:], st_out[g][:])
                        _bn_coeffs(nc, const, stga, g1t[:, a0 : a0 + HGRP],
                                   be1t[:, a0 : a0 + HGRP], HGRP, P,
                                   f"bn1g{g}",
                                   dsc1[:, a0 : a0 + HGRP],
                                   dsh1[:, a0 : a0 + HGRP])
                        if g < NGRP - 1:
                            lif_pending.extend(range(a0, a0 + HGRP))

                    # drain one pending LIF h-tile per step (one stats
                    # group behind, so the DVE work hides under FC1)
                    if a >= HGRP and lif_pending:
                        ap_ = lif_pending.pop(0)
                        _lif1_slice(nc, plif, y1[ap_], s1[ap_], ap_,
                                    dsc1, dsh1, 0, MLOC, "l1")
                        y1[ap_] = None

                while lif_pending:
                    ap_ = lif_pending.pop(0)
                    _lif1_slice(nc, plif, y1[ap_], s1[ap_], ap_,
                                dsc1, dsh1, 0, MLOC, "l1")
                    y1[ap_] = None

                # last h-group's LIF: first m-tile sliced out so FC2 can
                # start right after the last stats allreduce lands
                a0 = (NGRP - 1) * HGRP
                for m0, mlen, tg in ((0, MT, "l1s"), (MT, MLOC - MT, "l1r")):
                    for a in range(a0, KH):
                        _lif1_slice(nc, plif, y1[a], s1[a], a,
                                    dsc1, dsh1, m0, mlen, tg)
            # pax/py1/plif/ps_mm closed: x, w1, y1 SBUF space is free now

            # ---- phase B: FC2 (s1-stationary, row-major y2) ------------
            with (
                tc.tile_pool(name="pbc", bufs=1) as pbc,
                tc.tile_pool(name="pb", bufs=3) as pb,
                tc.tile_pool(name="ps2", bufs=5, space="PSUM") as ps2p,
                tc.tile_pool(name="ps_st", bufs=1, space="PSUM") as ps_st,
            ):
                w2h = pbc.tile([P, KH, C], F16)
                nc.sync.dma_start(w2h[:],
                                  w2h_d.ap().rearrange("k p c -> p k c"))
                w2l = pbc.tile([P, KH, C], F16)
                nc.sync.dma_start(w2l[:],
                                  w2l_d.ap().rearrange("k p c -> p k c"))

                acc_s = pbc.tile([P, C], F32)
                acc_q = pbc.tile([P, C], F32)
                nc.vector.memset(acc_s[:], 0.0)
                nc.vector.memset(acc_q[:], 0.0)

                # y2 output groups: (jb, t) full 128-row tiles plus a
                # 16-row group per t for the tail lanes
                y2 = {}
                s1f = [s1[k][:].rearrange("p t m -> p (t m)")
                       for k in range(KH)]
                fgroups = [(jb, t, MT) for jb in range(NMT)
                           for t in range(T)]
                fgroups += [(NMT, t, MTAIL) for t in range(T)]
                for jb, t, mlen in fgroups:
                    m0 = jb * MT
                    ps2 = ps2p.tile([P, C], F32, tag="mm2",
                                    name=f"ps2_{jb}_{t}")
                    idx = 0
                    for k in range(KH):
                        for wsp in (w2h, w2l):
                            nc.tensor.matmul(
                                ps2[:mlen, :],
                                s1f[k][:, t * MLOC + m0 :
                                       t * MLOC + m0 + mlen],
                                wsp[:, k, :],
                                start=(idx == 0),
                                stop=(idx == 2 * KH - 1),
                            )
                            idx += 1
                    y2t = pbc.tile([P, C], F32, tag=f"y2_{jb}_{t}",
                                   name=f"y2_{jb}_{t}")
                    y2[(jb, t)] = y2t
                    nc.scalar.activation(y2t[:mlen, :], ps2[:mlen, :],
                                         AF.Identity, bias=0.0, scale=1.0)
                    sq2 = pb.tile([P, C], F32, tag="sq2")
                    nc.scalar.activation(sq2[:mlen, :], ps2[:mlen, :],
                                         AF.Square, bias=0.0, scale=1.0)
                    nc.vector.tensor_tensor(acc_s[:mlen, :], acc_s[:mlen, :],
                                            y2t[:mlen, :], OP.add)
                    nc.vector.tensor_tensor(acc_q[:mlen, :], acc_q[:mlen, :],
                                            sq2[:mlen, :], OP.add)

                # BN2 stats: partition-reduce via ones-matmul, allreduce
                stp_s = ps_st.tile([1, C], F32, tag="sts")
                nc.tensor.matmul(stp_s[:], onesc[:], acc_s[:],
                                 start=True, stop=True)
                stp_q = ps_st.tile([1, C], F32, tag="stq")
                nc.tensor.matmul(stp_q[:], onesc[:], acc_q[:],
                                 start=True, stop=True)
                st2 = pbc.tile([1, 2 * C], F32)
                nc.vector.tensor_copy(st2[:, 0:C], stp_s[:])
                nc.vector.tensor_copy(st2[:, C : 2 * C], stp_q[:])
                nc.gpsimd.dma_start(st2_in[:], st2[:])
                nc.gpsimd.collective_compute(
                    "AllReduce", OP.add, replica_groups=groups,
                    ins=[st2_in.opt()], outs=[st2_out.opt()])
                stg2 = pbc.tile([1, 2 * C], F32)
                nc.gpsimd.dma_start(stg2[:], st2_out[:])
                dsc2 = pbc.tile([1, C], F32)
                dsh2 = pbc.tile([1, C], F32)
                _bn_coeffs(nc, pbc, stg2, g2r[:], be2r[:], C, 1, "bn2",
                           dsc2[:], dsh2[:])

                # broadcast [1, C] coeffs to all 128 partitions via PE
                dscB = pbc.tile([P, C], F32)
                dshB = pbc.tile([P, C], F32)
                for src, dst in ((dsc2, dscB), (dsh2, dshB)):
                    bps = ps_st.tile([P, C], F32, tag="bc")
                    nc.tensor.matmul(bps[:], ones1r[:], src[:],
                                     start=True, stop=True)
                    nc.vector.tensor_copy(dst[:], bps[:])

                # ---- phase C: BN2 affine + LIF2 + direct row-major out --
                for jb in range(NMT + 1):
                    mlen = MT if jb < NMT else MTAIL
                    v = None
                    for t in range(T):
                        y2t = y2[(jb, t)]
                        nc.vector.tensor_tensor(
                            y2t[:mlen, :], y2t[:mlen, :], dscB[:mlen, :],
                            OP.mult)
                        nc.vector.tensor_tensor(
                            y2t[:mlen, :], y2t[:mlen, :], dshB[:mlen, :],
                            OP.add)
                        d = y2t[:mlen, :]
                        if v is None:
                            h = d
                        else:
                            ht = pb.tile([P, C], F32, tag="l2_h",
                                         name=f"l2h{jb}_{t}")
                            nc.vector.scalar_tensor_tensor(
                                ht[:mlen, :], v, 0.5, d, OP.mult, OP.add)
                            h = ht[:mlen, :]
                        ob = pb.tile([P, C], F32, tag="ob",
                                     name=f"ob{jb}_{t}")
                        nc.vector.tensor_scalar(ob[:mlen, :], h, 1.0, None,
                                                OP.is_ge)
                        if t < T - 1:
                            vt = pb.tile([P, C], F32, tag="l2_v",
                                         name=f"l2v{jb}_{t}")
                            nc.vector.scalar_tensor_tensor(
                                vt[:mlen, :], h, 1.0, h, OP.is_lt, OP.mult)
                            v = vt[:mlen, :]
                        r0 = t * MLOC + jb * MT
                        nc.sync.dma_start(out_d[r0 : r0 + mlen, :],
                                          ob[:mlen, :])

    nc.compile()
    return nc


_NC = None
TRACE = False          # set by test harness to capture an NTFF profile
LAST_RESULT = None     # BassKernelResults of the most recent run


def _get_nc():
    global _NC
    if _NC is None:
        _NC = _build()
    return _NC


def _split_f16(a):
    hi = a.astype(np.float16)
    lo = (a - hi.astype(np.float32)).astype(np.float16)
    return np.ascontiguousarray(hi), np.ascontiguousarray(lo)


def _in_maps(x, W1, b1, g1, be1, W2, b2, g2, be2):
    x = np.asarray(x, dtype=np.float32)
    w1t = np.asarray(W1, np.float32).T.reshape(KC, P, H)
    w1thi, w1tlo = _split_f16(w1t)
    w2t = np.asarray(W2, np.float32).T.reshape(KH, P, C)
    w2thi, w2tlo = _split_f16(w2t)
    shared = {
        "w1thi": w1thi, "w1tlo": w1tlo,
        "w2thi": w2thi, "w2tlo": w2tlo,
        "g1": np.asarray(g1, np.float32),
        "be1": np.asarray(be1, np.float32),
        "g2": np.asarray(g2, np.float32),
        "be2": np.asarray(be2, np.float32),
    }
    in_maps = []
    for i in range(NCORES):
        xt = x[:, i * BLOC : (i + 1) * BLOC].reshape(R, C).T.reshape(KC, P, R)
        xthi, xtlo = _split_f16(xt)
        in_maps.append({"xthi": xthi, "xtlo": xtlo, **shared})
    return in_maps


def kernel(x, W1, b1, g1, be1, W2, b2, g2, be2):
    nc = _get_nc()
    in_maps = _in_maps(x, W1, b1, g1, be1, W2, b2, g2, be2)
    res = run_bass_kernel_spmd(nc, in_maps, core_ids=list(range(NCORES)),
                               trace=TRACE)
    global LAST_RESULT
    LAST_RESULT = res
    out = np.concatenate(
        [res.results[i]["out"].reshape(T, BLOC, NN, C) for i in range(NCORES)],
        axis=1,
    )
    return out


# revision 8
# speedup vs baseline: 1.0895x; 1.0895x over previous
"""Trainium2 Bass kernel for the DEER-MLP spiking network (v3).

Network: x(4,32,196,384) -> FC1(384->1536) -> BatchNorm -> LIF(T=4) ->
FC2(1536->384) -> BatchNorm -> LIF -> spikes(4,32,196,384).

Math notes:
 - The reference's 10 DEER Newton iterations over T=4 steps converge to the
   exact sequential LIF recurrence; we compute that directly.
 - The pre-BN biases b1/b2 cancel inside BatchNorm (additive per-channel
   constants are removed by the mean subtraction), so they are dropped.
 - Both matmuls run as multi-pass fp16 with hi/lo fp16 limbs (exact to
   ~2^-22): FC1 = x_hi@w_hi + x_lo@w_hi + x_hi@w_lo; FC2 spikes are exact
   in fp16 so two passes (w_hi + w_lo) suffice.

Distribution: data-parallel over B across 8 cores (784 lanes/core).
BatchNorm statistics are the only cross-core coupling: a warmup dummy
AllReduce (absorbs the runtime's ~50us CC-stream init barrier), four
staged BN1 stat AllReduces (pipelined under FC1), and one BN2 AllReduce.

v3 schedule:
 - FC1 is h-tile-outer (stationary weights, 7 uniform 448-col moving
   chunks).  Each finished y1 h-tile spills to DRAM through a small
   bounce ring, so FC1 never stalls on BN1-stats latency; per-chunk sum
   (DVE reduce) and sumsq (Square written back into the dead PSUM bank
   with accum_out) ride the evacuation.
 - BN1 stats AllReduce in 4 groups of 3 h-tiles, under FC1.  y1 h-tiles
   reload from DRAM one stats-group behind and run BN1 affine + LIF1 on
   DVE, all in the shadow of FC1; the last group is sliced so FC2 can
   start right after the final stats AllReduce.
 - FC2 uses s1 tiles as the stationary operand, producing y2 ROW-major;
   w2 is prefetched at kernel start (own pool - no WAR on freed space).
   BN2 stats: DVE accumulate + ones-matmul partition reduce; coeffs
   broadcast back to 128 partitions with a rank-1 matmul.  LIF2 emits
   spikes row-major and DMAs straight out - no transpose phase.
"""

import numpy as np

import concourse.bass as bass
import concourse.mybir as mybir
import concourse.tile as tile
from concourse import bacc
from concourse.bass_utils import run_bass_kernel_spmd

F32 = mybir.dt.float32
F16 = mybir.dt.float16
AF = mybir.ActivationFunctionType
OP = mybir.AluOpType
AX = mybir.AxisListType

T, B, NN, C, H = 4, 32, 196, 384, 1536
NCORES = 8
BLOC = B // NCORES            # 4 batches per core
MLOC = BLOC * NN              # 784 lanes per core
R = T * MLOC                  # 3136 flattened (t, m) rows per core
NTOT = T * B * NN             # 25088 batchnorm samples per channel
KC = C // 128                 # 3 c-tiles
KH = H // 128                 # 12 h-tiles
EPS = 1e-5
P = 128

CH = 448                      # FC1 moving-operand chunk (7 * 448 = 3136)
NCH = R // CH
MT = 128                      # FC2 m-tile (rows per output group)
NMT = MLOC // MT              # 6 full m-tiles per t step
MTAIL = MLOC - NMT * MT       # 16 tail rows per t step
HGRP = 3                      # h-tiles per BN1 stats allreduce group
NGRP = KH // HGRP             # 4 staged allreduces
LSL = MLOC // 2               # LIF1 slice width (full tiles)


def _lif1_slice(nc, pool, y1a, s1a, a, dsc1, dsh1, m0, mlen, tg):
    """BN1 affine + LIF over [P, T, m0:m0+mlen] of h-tile a (in place on
    y1a); spikes to s1a fp16.  tg selects the scratch-tile tag set."""
    yv = y1a[:].rearrange("p (t m) -> p t m", t=T)
    nc.vector.tensor_scalar(
        yv[:, :, m0 : m0 + mlen], yv[:, :, m0 : m0 + mlen],
        dsc1[:, a : a + 1], dsh1[:, a : a + 1], OP.mult, OP.add)
    v = None
    for t in range(T):
        d = yv[:, t, m0 : m0 + mlen]
        if v is None:
            h = d
        else:
            ht = pool.tile([P, mlen], F32, tag=f"{tg}_h", name=f"{tg}h{a}_{t}")
            nc.vector.scalar_tensor_tensor(ht[:], v, 0.5, d, OP.mult, OP.add)
            h = ht[:]
        nc.vector.tensor_scalar(s1a[:, t, m0 : m0 + mlen], h, 1.0, None,
                                OP.is_ge)
        if t < T - 1:
            vt = pool.tile([P, mlen], F32, tag=f"{tg}_v", name=f"{tg}v{a}_{t}")
            nc.vector.scalar_tensor_tensor(vt[:], h, 1.0, h, OP.is_lt,
                                           OP.mult)
            v = vt[:]


def _bn_coeffs(nc, pool, stg, gt2, bet2, k, pp, tag, dsc_out, dsh_out):
    """From allreduced [pp, 2k] (sum || sumsq) write the fused affine
    coeffs: drive = y*dsc + dsh == 0.5*((y - mean)*rsqrt(var+eps)*g + be).
    gt2/bet2 must be pre-scaled by 0.5."""
    mean = pool.tile([pp, k], F32, tag=f"{tag}_mean", name=f"{tag}_mean")
    nc.vector.tensor_scalar(mean[:], stg[:, 0:k], 1.0 / NTOT, None, OP.mult)
    msq = pool.tile([pp, k], F32, tag=f"{tag}_msq", name=f"{tag}_msq")
    nc.vector.tensor_tensor(msq[:], mean[:], mean[:], OP.mult)
    var = pool.tile([pp, k], F32, tag=f"{tag}_var", name=f"{tag}_var")
    nc.vector.scalar_tensor_tensor(var[:], stg[:, k : 2 * k], 1.0 / NTOT,
                                   msq[:], OP.mult, OP.subtract)
    nc.vector.tensor_scalar(var[:], var[:], EPS, None, OP.add)
    std = pool.tile([pp, k], F32, tag=f"{tag}_std", name=f"{tag}_std")
    nc.scalar.activation(std[:], var[:], AF.Sqrt, bias=0.0, scale=1.0)
    rstd = pool.tile([pp, k], F32, tag=f"{tag}_rstd", name=f"{tag}_rstd")
    nc.vector.reciprocal(rstd[:], std[:])
    nc.vector.tensor_tensor(dsc_out, rstd[:], gt2, OP.mult)
    nc.vector.tensor_tensor(dsh_out, mean[:], dsc_out, OP.mult)
    nc.vector.tensor_tensor(dsh_out, bet2, dsh_out, OP.subtract)


def _build():
    nc = bacc.Bacc("TRN2", target_bir_lowering=False, debug=False,
                   num_devices=NCORES)

    xh_d = nc.dram_tensor("xthi", [KC, P, R], F16, kind="ExternalInput")
    xl_d = nc.dram_tensor("xtlo", [KC, P, R], F16, kind="ExternalInput")
    w1h_d = nc.dram_tensor("w1thi", [KC, P, H], F16, kind="ExternalInput")
    w1l_d = nc.dram_tensor("w1tlo", [KC, P, H], F16, kind="ExternalInput")
    w2h_d = nc.dram_tensor("w2thi", [KH, P, C], F16, kind="ExternalInput")
    w2l_d = nc.dram_tensor("w2tlo", [KH, P, C], F16, kind="ExternalInput")
    g1_d = nc.dram_tensor("g1", [H], F32, kind="ExternalInput")
    be1_d = nc.dram_tensor("be1", [H], F32, kind="ExternalInput")
    g2_d = nc.dram_tensor("g2", [C], F32, kind="ExternalInput")
    be2_d = nc.dram_tensor("be2", [C], F32, kind="ExternalInput")
    out_d = nc.dram_tensor("out", [R, C], F32, kind="ExternalOutput")

    groups = [list(range(NCORES))]

    with tile.TileContext(nc) as tc:
        with (
            tc.tile_pool(name="const", bufs=1) as const,
            tc.tile_pool(name="dram", bufs=1, space="DRAM") as dram,
            tc.tile_pool(name="pw2", bufs=1) as pw2,
        ):
            def colvec(dst_k, src, half=False):
                t_ = const.tile([P, dst_k], F32, name=f"cv_{src.name}",
                                tag=f"cv_{src.name}")
                nc.sync.dma_start(
                    t_[:], src.ap().rearrange("(a p) -> p a", p=P))
                if half:
                    nc.vector.tensor_scalar(t_[:], t_[:], 0.5, None, OP.mult)
                return t_

            def rowvec(src, n, half=False):
                t_ = const.tile([1, n], F32, name=f"rv_{src.name}",
                                tag=f"rv_{src.name}")
                nc.sync.dma_start(t_[:], src.ap().rearrange("(a c) -> a c",
                                                            a=1))
                if half:
                    nc.vector.tensor_scalar(t_[:], t_[:], 0.5, None, OP.mult)
                return t_

            g1t, be1t = colvec(KH, g1_d, True), colvec(KH, be1_d, True)
            g2r, be2r = rowvec(g2_d, C, True), rowvec(be2_d, C, True)

            onesc = const.tile([P, 1], F32)
            nc.vector.memset(onesc[:], 1.0)
            ones1r = const.tile([1, P], F32)
            nc.vector.memset(ones1r[:], 1.0)

            # Warmup collective: absorbs the CC-stream init barrier +
            # launch skew in the shadow of the input DMAs.  Its result is
            # consumed (times zero) much later so it can't be DCE'd and
            # its load-back doesn't block the stats DMAs.
            bar_s = const.tile([1, 8], F32)
            nc.vector.memset(bar_s[:], 1.0)
            bar_in = dram.tile([1, 8], F32, tag="bar_in", name="bar_in")
            bar_out = dram.tile([1, 8], F32, tag="bar_out", name="bar_out")
            nc.gpsimd.dma_start(bar_in[:], bar_s[:])
            nc.gpsimd.collective_compute(
                "AllReduce", OP.add, replica_groups=groups,
                ins=[bar_in.opt()], outs=[bar_out.opt()])

            # persistent across phases
            s1 = [const.tile([P, T, MLOC], F16, tag=f"s1_{k}",
                             name=f"s1_{k}") for k in range(KH)]
            asum1 = const.tile([P, KH, NCH], F32)
            asq1 = const.tile([P, KH, NCH], F32)
            dsc1 = const.tile([P, KH], F32)
            dsh1 = const.tile([P, KH], F32)

            y1d = [dram.tile([P, R], F32, tag=f"y1d{a}", name=f"y1d{a}")
                   for a in range(KH)]
            st_in = [dram.tile([P, 2 * HGRP], F32, tag=f"sti{g}",
                               name=f"sti{g}") for g in range(NGRP)]
            st_out = [dram.tile([P, 2 * HGRP], F32, tag=f"sto{g}",
                                name=f"sto{g}") for g in range(NGRP)]
            st2_in = dram.tile([1, 2 * C], F32)
            st2_out = dram.tile([1, 2 * C], F32)

            # ---- phase A: FC1 (h-outer) + staged BN1 stats + LIF1 ------
            with (
                tc.tile_pool(name="pax", bufs=1) as pax,
                tc.tile_pool(name="pbn", bufs=3) as pbn,
                tc.tile_pool(name="prel", bufs=3) as prel,
                tc.tile_pool(name="plif", bufs=2) as plif,
                tc.tile_pool(name="ps_mm", bufs=1, space="PSUM") as ps_mm,
            ):
                # input DMAs, ordered by first use; w2 prefetches into its
                # own long-lived pool so nothing makes it wait on freed
                # space at the phase boundary.
                w1h = pax.tile([P, KC, H], F16)
                nc.sync.dma_start(w1h[:],
                                  w1h_d.ap().rearrange("k p h -> p k h"))
                xh = pax.tile([P, KC, R], F16)
                nc.sync.dma_start(xh[:], xh_d.ap().rearrange("k p r -> p k r"))
                w1l = pax.tile([P, KC, H], F16)
                nc.sync.dma_start(w1l[:],
                                  w1l_d.ap().rearrange("k p h -> p k h"))
                xl = pax.tile([P, KC, R], F16)
                nc.sync.dma_start(xl[:], xl_d.ap().rearrange("k p r -> p k r"))
                w2h = pw2.tile([P, KH, C], F16)
                nc.sync.dma_start(w2h[:],
                                  w2h_d.ap().rearrange("k p c -> p k c"))
                w2l = pw2.tile([P, KH, C], F16)
                nc.sync.dma_start(w2l[:],
                                  w2l_d.ap().rearrange("k p c -> p k c"))

                rel = [None] * KH

                def ensure_rel(a_):
                    if rel[a_] is None:
                        rt = prel.tile([P, R], F32, tag="rel",
                                       name=f"rel{a_}")
                        nc.sync.dma_start(rt[:], y1d[a_][:])
                        rel[a_] = rt

                def reload_lif(a_, m0, mlen, tg):
                    ensure_rel(a_)
                    _lif1_slice(nc, plif, rel[a_], s1[a_], a_,
                                dsc1, dsh1, m0, mlen, tg)

                lif_pending = []
                for a in range(KH):
                    pss = [ps_mm.tile([P, CH], F32, tag=f"mm{c}",
                                      name=f"ps{a}_{c}") for c in range(NCH)]
                    idx = 0
                    for wt, xt in ((w1h, xh), (w1l, xh), (w1h, xl)):
                        for k in range(KC):
                            for c in range(NCH):
                                nc.tensor.matmul(
                                    pss[c][:],
                                    wt[:, k, a * P : (a + 1) * P],
                                    xt[:, k, c * CH : (c + 1) * CH],
                                    start=(idx == 0),
                                    stop=(idx == 8),
                                )
                            idx += 1
                    for c in range(NCH):
                        # evacuate through a bounce ring to DRAM; sum via
                        # DVE reduce, sumsq via a Square written back into
                        # the (dead) psum bank with accum_out
                        bt = pbn.tile([P, CH], F32, tag="bn",
                                      name=f"bn{a}_{c}")
                        nc.scalar.activation(bt[:], pss[c][:], AF.Identity,
                                             bias=0.0, scale=1.0)
                        nc.scalar.activation(
                            pss[c][:], pss[c][:], AF.Square,
                            bias=0.0, scale=1.0,
                            accum_out=asq1[:, a, c : c + 1])
                        nc.vector.tensor_reduce(
                            asum1[:, a, c : c + 1], bt[:], AX.X, OP.add)
                        nc.sync.dma_start(y1d[a][:, c * CH : (c + 1) * CH],
                                          bt[:])

                    if a % HGRP == HGRP - 1:
                        g = a // HGRP
                        a0 = g * HGRP
                        stg = const.tile([P, 2 * HGRP], F32,
                                         tag=f"stg{g}", name=f"stg{g}")
                        nc.vector.tensor_reduce(
                            stg[:, 0:HGRP],
                            asum1[:, a0 : a0 + HGRP, :], AX.X, OP.add)
                        nc.vector.tensor_reduce(
                            stg[:, HGRP : 2 * HGRP],
                            asq1[:, a0 : a0 + HGRP, :], AX.X, OP.add)
                        nc.gpsimd.dma_start(st_in[g][:], stg[:])
                        nc.gpsimd.collective_compute(
                            "AllReduce", OP.add, replica_groups=groups,
                            ins=[st_in[g].opt()], outs=[st_out[g].opt()])
                        stga = const.tile([P, 2 * HGRP], F32,
                                          tag=f"stga{g}", name=f"stga{g}")
                        nc.gpsimd.dma_start(stga[:], st_out[g][:])
                        _bn_coeffs(nc, const, stga, g1t[:, a0 : a0 + HGRP],
                                   be1t[:, a0 : a0 + HGRP], HGRP, P,
                                   f"bn1g{g}",
                                   dsc1[:, a0 : a0 + HGRP],
                                   dsh1[:, a0 : a0 + HGRP])
                        if g < NGRP - 1:
                            lif_pending.extend(range(a0, a0 + HGRP))

                    # drain one pending LIF h-tile per step (one stats
                    # group behind, so the DVE work hides under FC1);
                    # prefetch the next reload so LIF never waits on DMA
                    if a >= HGRP and lif_pending:
                        ap_ = lif_pending.pop(0)
                        ensure_rel(ap_)
                        if lif_pending:
                            ensure_rel(lif_pending[0])
                        for m0 in range(0, MLOC, LSL):
                            reload_lif(ap_, m0, LSL, "l1")

                while lif_pending:
                    ap_ = lif_pending.pop(0)
                    for m0 in range(0, MLOC, LSL):
                        reload_lif(ap_, m0, LSL, "l1")

                # last h-group: first m-tile sliced out for every h so FC2
                # can start right after the last stats allreduce lands
                a0 = (NGRP - 1) * HGRP
                for a in range(a0, KH):
                    ensure_rel(a)
                for m0, mlen, tg in ((0, MT, "l1s"), (MT, LSL - MT, "l1r"),
                                     (LSL, LSL - MT, "l1r"),
                                     (MLOC - MT, MT, "l1s")):
                    for a in range(a0, KH):
                        reload_lif(a, m0, mlen, tg)

            # ---- phase B: FC2 (s1-stationary, row-major y2) ------------
            with (
                tc.tile_pool(name="pbc", bufs=1) as pbc,
                tc.tile_pool(name="pb", bufs=3) as pb,
                tc.tile_pool(name="ps2", bufs=5, space="PSUM") as ps2p,
                tc.tile_pool(name="ps_st", bufs=1, space="PSUM") as ps_st,
            ):
                acc_s = pbc.tile([P, C], F32)
                acc_q = pbc.tile([P, C], F32)
                nc.vector.memset(acc_s[:], 0.0)
                nc.vector.memset(acc_q[:], 0.0)

                # y2 output groups: (jb, t) full 128-row tiles plus a
                # 16-row group per t for the tail lanes
                y2 = {}
                s1f = [s1[k][:].rearrange("p t m -> p (t m)")
                       for k in range(KH)]
                fgroups = [(jb, t, MT) for jb in range(NMT)
                           for t in range(T)]
                fgroups += [(NMT, t, MTAIL) for t in range(T)]
                for jb, t, mlen in fgroups:
                    m0 = jb * MT
                    ps2 = ps2p.tile([P, C], F32, tag="mm2",
                                    name=f"ps2_{jb}_{t}")
                    idx = 0
                    for k in range(KH):
                        for wsp in (w2h, w2l):
                            nc.tensor.matmul(
                                ps2[:mlen, :],
                                s1f[k][:, t * MLOC + m0 :
                                       t * MLOC + m0 + mlen],
                                wsp[:, k, :],
                                start=(idx == 0),
                                stop=(idx == 2 * KH - 1),
                            )
                            idx += 1
                    y2t = pbc.tile([P, C], F32, tag=f"y2_{jb}_{t}",
                                   name=f"y2_{jb}_{t}")
                    y2[(jb, t)] = y2t
                    nc.scalar.activation(y2t[:mlen, :], ps2[:mlen, :],
                                         AF.Identity, bias=0.0, scale=1.0)
                    sq2 = pb.tile([P, C], F32, tag="sq2")
                    nc.scalar.activation(sq2[:mlen, :], ps2[:mlen, :],
                                         AF.Square, bias=0.0, scale=1.0)
                    nc.vector.tensor_tensor(acc_s[:mlen, :], acc_s[:mlen, :],
                                            y2t[:mlen, :], OP.add)
                    nc.vector.tensor_tensor(acc_q[:mlen, :], acc_q[:mlen, :],
                                            sq2[:mlen, :], OP.add)

                # BN2 stats: partition-reduce via ones-matmul, allreduce
                stp_s = ps_st.tile([1, C], F32, tag="sts")
                nc.tensor.matmul(stp_s[:], onesc[:], acc_s[:],
                                 start=True, stop=True)
                stp_q = ps_st.tile([1, C], F32, tag="stq")
                nc.tensor.matmul(stp_q[:], onesc[:], acc_q[:],
                                 start=True, stop=True)
                # consume the warmup-barrier output here (cheap, off the
                # critical path, keeps it from being dead-code-eliminated)
                bar_r = pbc.tile([1, 8], F32)
                nc.gpsimd.dma_start(bar_r[:], bar_out[:])
                st2 = pbc.tile([1, 2 * C], F32)
                nc.vector.tensor_copy(st2[:, 0:C], stp_s[:])
                nc.vector.tensor_copy(st2[:, C : 2 * C], stp_q[:])
                nc.vector.scalar_tensor_tensor(
                    st2[0:1, 0:8], bar_r[:], 0.0, st2[0:1, 0:8],
                    OP.mult, OP.add)
                nc.gpsimd.dma_start(st2_in[:], st2[:])
                nc.gpsimd.collective_compute(
                    "AllReduce", OP.add, replica_groups=groups,
                    ins=[st2_in.opt()], outs=[st2_out.opt()])
                stg2 = pbc.tile([1, 2 * C], F32)
                nc.gpsimd.dma_start(stg2[:], st2_out[:])
                dsc2 = pbc.tile([1, C], F32)
                dsh2 = pbc.tile([1, C], F32)
                _bn_coeffs(nc, pbc, stg2, g2r[:], be2r[:], C, 1, "bn2",
                           dsc2[:], dsh2[:])

                # broadcast [1, C] coeffs to all 128 partitions via PE
                dscB = pbc.tile([P, C], F32)
                dshB = pbc.tile([P, C], F32)
                for src, dst in ((dsc2, dscB), (dsh2, dshB)):
                    bps = ps_st.tile([P, C], F32, tag="bc")
                    nc.tensor.matmul(bps[:], ones1r[:], src[:],
                                     start=True, stop=True)
                    nc.vector.tensor_copy(dst[:], bps[:])

                # ---- phase C: BN2 affine + LIF2 + direct row-major out --
                for jb in range(NMT + 1):
                    mlen = MT if jb < NMT else MTAIL
                    v = None
                    for t in range(T):
                        y2t = y2[(jb, t)]
                        nc.vector.tensor_tensor(
                            y2t[:mlen, :], y2t[:mlen, :], dscB[:mlen, :],
                            OP.mult)
                        nc.vector.tensor_tensor(
                            y2t[:mlen, :], y2t[:mlen, :], dshB[:mlen, :],
                            OP.add)
                        d = y2t[:mlen, :]
                        if v is None:
                            h = d
                        else:
                            ht = pb.tile([P, C], F32, tag="l2_h",
                                         name=f"l2h{jb}_{t}")
                            nc.vector.scalar_tensor_tensor(
                                ht[:mlen, :], v, 0.5, d, OP.mult, OP.add)
                            h = ht[:mlen, :]
                        ob = pb.tile([P, C], F32, tag="ob",
                                     name=f"ob{jb}_{t}")
                        nc.vector.tensor_scalar(ob[:mlen, :], h, 1.0, None,
                                                OP.is_ge)
                        if t < T - 1:
                            vt = pb.tile([P, C], F32, tag="l2_v",
                                         name=f"l2v{jb}_{t}")
                            nc.vector.scalar_tensor_tensor(
                                vt[:mlen, :], h, 1.0, h, OP.is_lt, OP.mult)
                            v = vt[:mlen, :]
                        r0 = t * MLOC + jb * MT
                        nc.sync.dma_start(out_d[r0 : r0 + mlen, :],
                                          ob[:mlen, :])

    nc.compile()
    return nc


_NC = None
TRACE = False          # set by test harness to capture an NTFF profile
LAST_RESULT = None     # BassKernelResults of the most recent run


def _get_nc():
    global _NC
    if _NC is None:
        _NC = _build()
    return _NC


def _split_f16(a):
    hi = a.astype(np.float16)
    lo = (a - hi.astype(np.float32)).astype(np.float16)
    return np.ascontiguousarray(hi), np.ascontiguousarray(lo)


def _in_maps(x, W1, b1, g1, be1, W2, b2, g2, be2):
    x = np.asarray(x, dtype=np.float32)
    w1t = np.asarray(W1, np.float32).T.reshape(KC, P, H)
    w1thi, w1tlo = _split_f16(w1t)
    w2t = np.asarray(W2, np.float32).T.reshape(KH, P, C)
    w2thi, w2tlo = _split_f16(w2t)
    shared = {
        "w1thi": w1thi, "w1tlo": w1tlo,
        "w2thi": w2thi, "w2tlo": w2tlo,
        "g1": np.asarray(g1, np.float32),
        "be1": np.asarray(be1, np.float32),
        "g2": np.asarray(g2, np.float32),
        "be2": np.asarray(be2, np.float32),
    }
    in_maps = []
    for i in range(NCORES):
        xt = x[:, i * BLOC : (i + 1) * BLOC].reshape(R, C).T.reshape(KC, P, R)
        xthi, xtlo = _split_f16(xt)
        in_maps.append({"xthi": xthi, "xtlo": xtlo, **shared})
    return in_maps


def kernel(x, W1, b1, g1, be1, W2, b2, g2, be2):
    nc = _get_nc()
    in_maps = _in_maps(x, W1, b1, g1, be1, W2, b2, g2, be2)
    res = run_bass_kernel_spmd(nc, in_maps, core_ids=list(range(NCORES)),
                               trace=TRACE)
    global LAST_RESULT
    LAST_RESULT = res
    out = np.concatenate(
        [res.results[i]["out"].reshape(T, BLOC, NN, C) for i in range(NCORES)],
        axis=1,
    )
    return out
